# revision 9
# baseline (speedup 1.0000x reference)
"""Trainium2 Bass kernel for nn_BmmEnsemble (ANI-style per-species ensemble MLP).

Math (see module reference): for each species s (4) and ensemble member e (8),
the species' atoms' AEV rows go through a 384->160->128->96->1 MLP with
CELU(0.1) after the first three layers; the output is the global sum over all
atoms of the ensemble-mean of the final scalar.

v2: fp8 DoubleRow pipeline.  celu(z,a) = a*elu(z/a), so the network is
rescaled so every activation is elu (alpha=1) exactly; layers 0/1 store
g = elu(u)+1 >= 0 and fold the "-1" into the next layer's bias.

All three matmul layers run as fp8 (e4m3) DoubleRow matmuls (2 fp8 K-planes
per instruction at 0.5 PE-cycles/output-column = 4x bf16 throughput):

 - L0 (K=384): x is quantized to e4m3 on the host (plus an x/64 copy).
   Weights are hi + 64*lo e4m3 pairs (residual compensation kills the
   systematic weight-quantization bias: measured 5.8e-3 -> 7.7e-5).
   3 DoubleRows per 128-col output tile: planes (k0h,k1h),(k2h,k0lo),
   (k1lo,k2lo).
 - L1 (K=160): per member one hi-e4m3 DoubleRow with planes
   (w1[0:128], g0a) and (zero-padded w1[128:160] tail, merged g0b), plus
   one e5m2 lo-residual DoubleRow on the same ifmaps (9.1e-3 -> 1.3e-3).
 - L2 (K=128): merged-bank layout (4 members' 96 outputs packed into
   3x128 psum rows per quad); each normal bank is ONE DoubleRow with
   planes (piece0, g1[m0]) and (piece1, g1[m1]).  Plain e4m3 (1.9e-4).

Elementwise is split across three engines (PSUM has one DVE read port, so
every psum pass runs at 1 elem/lane/cycle; the split is the only lever):
 - L0 and most L1 activations: single-pass custom-DVE poly
       elu(u)+1 ~= max(u + 1, clamp(1 + k*u, 0, 1)^4)       (k = 0.21)
   writing e4m3 directly.
 - SCAL1 members' L1 and ALL of L2: EXACT elu+1 = r + m split as
   rho = Relu(-u-b) (ScalarE, psum), m = Exp(-rho) (ScalarE, sbuf),
   r = max(u+b, 0) (GPSIMD scalar_tensor_tensor, psum).  The consuming
   DoubleRow takes (r, m) as its two planes with the same weights, so
   r+m never needs an add pass.  For L2 the m and r passes write their
   row-sums through the hardware accumulators (accum_out) directly into
   the rs_m / rs_r outputs - no DVE pass at all, and L2 is exact.

Distribution: data-parallel over atoms (2048 atoms/species/core), per-species
weights replicated, host applies the tiny w3 dot and sums the per-core
row-sum outputs (the "all-reduce").  Expected end-to-end error ~3e-4 from
host emulation (gate 2e-2).
"""

import os

import numpy as np

import concourse.dve_ops as _dve_ops
import concourse.mybir as mybir
import concourse.tile as tile
from concourse import bacc
from concourse.bass_utils import run_bass_kernel_spmd
from operator import add as _operator_add

from concourse.dve_spec import (
    C0,
    C1,
    C2,
    One,
    Spec,
    Src0,
    Src1,
    Zero,
    _has_src1,
    lower,
    maxx,
    minn,
    relu,
    sq,
)
from concourse.dve_uop import DveOpSpec

# ---------------------------------------------------------------- constants
S, E = 4, 8
N_ATOMS = 65536
N_CORES = 8
A_SP = N_ATOMS // S // N_CORES      # atoms per species per core = 2048
CHUNK = 512
NCH = A_SP // CHUNK                 # 4 chunks
K0, H0, H1, H2 = 384, 160, 128, 96
KT = K0 // 128                      # 3 K-tiles for layer 0
NP0 = 3                             # DoubleRow pairs for layer 0 (hi+lo)
NQ = 2                              # member quads per species (E/4)
ALPHA = 0.1
KP = 0.21                           # (1 + KP*u)^4 ~ e^u
LO_SCALE = 64.0                     # w0 residual scale
SCAL1 = ()                          # members-within-quad on the exact L1 path
SCALA_BANKS = (0, 1, 2)             # L2 banks on the ScalarE 3-pass accum path

F32 = mybir.dt.float32
F32R = mybir.dt.float32r
BF16 = mybir.dt.bfloat16
F8 = mybir.dt.float8e4
F8E5 = mybir.dt.float8e5
DR = mybir.MatmulPerfMode.DoubleRow
EXP = mybir.ActivationFunctionType.Exp
RELU = mybir.ActivationFunctionType.Relu
ADD = mybir.AluOpType.add
MUL = mybir.AluOpType.mult
MAX = mybir.AluOpType.max

# ------------------------------------------------------- custom DVE op
# POLY_ELU4: out = max(z + C1, clamp(z*C2 + C0, 0, 1)^4)  ==  elu(u)+1 approx
# with u = z + b;  C0 = 1 + k*b (per-partition), C1 = b + 1, C2 = k.
_B_POLY = maxx(Src0 + C1, sq(sq(minn(relu(Src0 * C2 + C0), One))))
# CELU blend (exact, alpha=1): in1 = exp(u) from ScalarE;
# out = relu(z + C0) + min(in1*C1 - C1, 0) = elu(u) for C1 = 1.
_B_BLEND = relu(Src0 + C0) + minn(Src1 * C1 - C1, Zero)


def _ref_poly(in0, in1, s0, s1, imm2):
    z = in0.astype(np.float32)
    s = np.minimum(np.maximum(z * imm2 + s0, 0.0), 1.0)
    return np.maximum(z + s1, (s * s) * (s * s)).astype(np.float32)


def _ref_blend_acc(in0, in1, s0, s1, imm2):
    z = in0.astype(np.float32) + s0
    b = (np.maximum(z, 0.0)
         + np.minimum(in1.astype(np.float32) * s1 - s1, 0.0)).astype(np.float32)
    return b, b.reshape(b.shape[0], -1).sum(axis=-1, keepdims=True)


def _mk_op(name, spec):
    row = _dve_ops._CUSTOM_DVE_ROW_BASE + len(_dve_ops.OPS)
    assert row < 0x20, "custom-DVE opcode rows exhausted"
    _dve_ops._SUB_OPCODE_FOR_NAME[name] = row
    shas = {}
    for ver in ("v3", "v4"):
        s = DveOpSpec(
            name=name, opcode=row, uops=lower(spec, ver=ver), rd1_en=_has_src1(spec)
        )
        shas[ver] = s.sha(ver)
    op = _dve_ops.DveOp(name, spec, subdim=False, uops_sha=shas)
    _dve_ops.OPS.append(op)
    _dve_ops.CUSTOM_DVE_SPECS[name] = spec
    return op


def _register_ops():
    existing = {o.name: o for o in _dve_ops.OPS}
    if "POLY_ELU4_ANT" in existing:
        return existing["POLY_ELU4_ANT"], existing["ELU_BLEND_ACC_ANT"]
    poly = _mk_op("POLY_ELU4_ANT", Spec(body=_B_POLY, reference=_ref_poly))
    blend = _mk_op(
        "ELU_BLEND_ACC_ANT",
        Spec(body=_B_BLEND, accum=_operator_add, accum_init=Zero,
             reference=_ref_blend_acc),
    )
    return poly, blend


# ------------------------------------------------------------ device build
_NC = None

# merged-L2 bank layout: per quad, (bank, piece) -> (member_in_quad,
# w2-col range, psum-row offset)
_L2_PIECES = [
    [(0, 0, 96, 0), (1, 0, 32, 96)],
    [(1, 32, 96, 0), (2, 0, 64, 64)],
    [(2, 64, 96, 0), (3, 0, 96, 32)],
]

# G1 slot layout within a quad: scal members own (r, m) slot pairs, poly
# members one slot.
_G1_SLOT = {}
_sl = 0
for _m in range(4):
    _G1_SLOT[_m] = _sl
    _sl += 2 if _m in SCAL1 else 1
G1_NSLOT = _sl


def _build_nc():
    global _NC
    if _NC is not None:
        return _NC
    POLY, BLEND_ACC = _register_ops()

    nc = bacc.Bacc("TRN2", target_bir_lowering=False, debug=False)

    # per-core inputs: fp8 feature-major atoms in DoubleRow plane pairs
    xq_d = nc.dram_tensor("xq", [S, 128, NP0, 2, A_SP], F8, kind="ExternalInput")
    # replicated weight packs
    w0a_d = nc.dram_tensor("w0a", [S, 128, NP0, 2, E * 128], F8, kind="ExternalInput")
    w0b_d = nc.dram_tensor("w0b", [S, 128, NP0, 2, NQ * 128], F8, kind="ExternalInput")
    w1h_d = nc.dram_tensor("w1h", [S, 128, 2, E * 128], F8, kind="ExternalInput")
    w1l_d = nc.dram_tensor("w1l", [S, 128, 2, E * 128], F8E5, kind="ExternalInput")
    w2p_d = nc.dram_tensor("w2p", [S, 128, 2, NQ * 3 * 128], F8, kind="ExternalInput")
    # bias packs; *_c0 = 1 + k*b (poly clamp offset), *_c1 = b + 1 (linear);
    # *_b / *_nb = plain / negated bias for the exact (scal) path.
    b0a_c0 = nc.dram_tensor("b0a_c0", [128, S * E], F32, kind="ExternalInput")
    b0a_c1 = nc.dram_tensor("b0a_c1", [128, S * E], F32, kind="ExternalInput")
    b0b_c0 = nc.dram_tensor("b0b_c0", [128, S * NQ], F32, kind="ExternalInput")
    b0b_c1 = nc.dram_tensor("b0b_c1", [128, S * NQ], F32, kind="ExternalInput")
    b1_c0 = nc.dram_tensor("b1_c0", [H1, S * E], F32, kind="ExternalInput")
    b1_c1 = nc.dram_tensor("b1_c1", [H1, S * E], F32, kind="ExternalInput")
    b1_b = nc.dram_tensor("b1_b", [H1, S * E], F32, kind="ExternalInput")
    b1_nb = nc.dram_tensor("b1_nb", [H1, S * E], F32, kind="ExternalInput")
    b2_d = nc.dram_tensor("b2_d", [128, S * NQ * 3], F32, kind="ExternalInput")
    b2_nb = nc.dram_tensor("b2_nb", [128, S * NQ * 3], F32, kind="ExternalInput")
    # outputs: blend banks write row-sums of elu(u2) into rs; scalA banks
    # write row-sums of r2 and m2 (elu+1 = r2+m2) into rs_r / rs_m.
    rs_d = nc.dram_tensor("rs", [128, S * NQ * 3 * NCH], F32, kind="ExternalOutput")
    rsr_d = nc.dram_tensor("rs_r", [128, S * NQ * 3 * NCH], F32, kind="ExternalOutput")
    rsm_d = nc.dram_tensor("rs_m", [128, S * NQ * 3 * NCH], F32, kind="ExternalOutput")

    with tile.TileContext(nc) as tc:
        with (
            tc.tile_pool(name="xp", bufs=2) as xp,
            tc.tile_pool(name="wp", bufs=2) as wp,
            tc.tile_pool(name="bp", bufs=1) as bp,
            tc.tile_pool(name="gp", bufs=2) as gp,
            tc.tile_pool(name="ep", bufs=3) as ep,
            tc.tile_pool(name="ps", bufs=2, space="PSUM") as psp,
        ):
            # warm the ACT Exp/Relu table during the initial DMA wait
            warm = bp.tile([1, 1], F32, tag="warm", name="warm")
            nc.vector.memset(warm[:], 0.0)
            nc.scalar.activation(warm[:], warm[:], EXP)


            B = {}
            _bias_dmas = []
            for nm, d, p in (
                ("b0a_c0", b0a_c0, 128), ("b0a_c1", b0a_c1, 128),
                ("b0b_c0", b0b_c0, 128), ("b0b_c1", b0b_c1, 128),
                ("b1_c0", b1_c0, H1), ("b1_c1", b1_c1, H1),
                ("b1_b", b1_b, H1), ("b1_nb", b1_nb, H1),
                ("b2_d", b2_d, 128), ("b2_nb", b2_nb, 128),
            ):
                t = bp.tile([p, d.shape[-1]], F32, tag=nm, name=nm)
                _bias_dmas.append((t, d))
                B[nm] = t
            RS = bp.tile([128, S * NQ * 3 * NCH], F32, tag="RS", name="RS")
            RSR = bp.tile([128, S * NQ * 3 * NCH], F32, tag="RSR", name="RSR")
            RSM = bp.tile([128, S * NQ * 3 * NCH], F32, tag="RSM", name="RSM")
            nc.vector.memset(RS[:], 0.0)
            nc.vector.memset(RSR[:], 0.0)
            nc.vector.memset(RSM[:], 0.0)

            for s in range(S):
                xt = xp.tile([128, NP0, 2, A_SP], F8, tag="x", name=f"x_{s}")
                # first-chunk x + all weights first so chunk-0 compute starts
                # as early as possible; remaining x chunks stream after.
                nc.sync.dma_start(xt[:, :, :, 0:CHUNK], xq_d[s, :, :, :, 0:CHUNK])
                w0at = wp.tile([128, NP0, 2, E * 128], F8, tag="w0a", name=f"w0a_{s}")
                nc.sync.dma_start(w0at[:], w0a_d[s])
                w0bt = wp.tile([128, NP0, 2, NQ * 128], F8, tag="w0b", name=f"w0b_{s}")
                nc.sync.dma_start(w0bt[:], w0b_d[s])
                if s == 0:
                    for t, d in _bias_dmas:
                        nc.sync.dma_start(t[:], d[:])
                w1ht = wp.tile([128, 2, E * 128], F8, tag="w1h", name=f"w1h_{s}")
                nc.sync.dma_start(w1ht[:], w1h_d[s])
                w1lt = wp.tile([128, 2, E * 128], F8E5, tag="w1l", name=f"w1l_{s}")
                nc.sync.dma_start(w1lt[:], w1l_d[s])
                w2pt = wp.tile([128, 2, NQ * 3 * 128], F8, tag="w2p", name=f"w2p_{s}")
                nc.sync.dma_start(w2pt[:], w2p_d[s])
                nc.sync.dma_start(
                    xt[:, :, :, CHUNK:A_SP], xq_d[s, :, :, :, CHUNK:A_SP]
                )

                for c in range(NCH):
                    cs = slice(c * CHUNK, (c + 1) * CHUNK)
                    for q in range(NQ):
                        sq_i = s * NQ + q
                        # ---- merged layer-0b for the 4 members of this quad
                        ps0b = psp.tile([128, CHUNK], F32, tag="l0b", bufs=1)
                        for p in range(NP0):
                            nc.tensor.matmul(
                                ps0b[:],
                                w0bt[:, p, :, q * 128:(q + 1) * 128],
                                xt[:, p, :, cs],
                                start=(p == 0),
                                stop=(p == NP0 - 1),
                                perf_mode=DR,
                            )
                        # G0: slots 0-3 = member g0a, slot 4 = merged g0b
                        G0 = gp.tile([128, 5, CHUNK], F8, tag="G0",
                                     name=f"G0_{s}_{c}_{q}")
                        nc.vector._custom_dve(
                            POLY, out=G0[:, 4, :], in0=ps0b[:],
                            s0=B["b0b_c0"][:, sq_i:sq_i + 1],
                            s1=B["b0b_c1"][:, sq_i:sq_i + 1], imm2=KP,
                        )
                        # G1: scal members own (r, m) slot pairs, poly one slot
                        G1 = gp.tile([128, G1_NSLOT, CHUNK], F8, tag="G1",
                                     name=f"G1_{s}_{c}_{q}")

                        def do_l2_bank(b):
                            (m0, _, _, _), (m1, _, _, _) = _L2_PIECES[b]
                            ps2 = psp.tile([128, CHUNK], F32, tag="l2",
                                           name=f"ps2_{b}", bufs=2)
                            off = (q * 3 + b) * 128
                            for piece, m in ((0, m0), (1, m1)):
                                sl = _G1_SLOT[m]
                                if m in SCAL1:
                                    # (r, m) planes, same weights
                                    nc.tensor.matmul(
                                        ps2[:],
                                        w2pt[:, piece:piece + 1, off:off + 128]
                                        .broadcast_to([128, 2, 128]),
                                        G1[:, sl:sl + 2, :],
                                        start=(piece == 0), stop=(piece == 1),
                                        perf_mode=DR,
                                    )
                                else:
                                    nc.tensor.matmul(
                                        ps2[:],
                                        w2pt[:, piece, off:off + 128],
                                        G1[:, sl, :],
                                        start=(piece == 0), stop=(piece == 1),
                                    )
                            sqb = (s * NQ + q) * 3 + b
                            col = sqb * NCH + c
                            if b in SCALA_BANKS:
                                # exact elu+1 = r2 + m2, pure ScalarE with
                                # hw-accumulated row-sums.
                                rho2 = ep.tile([128, CHUNK], BF16, tag="rho2",
                                               name=f"rho2_{b}")
                                nc.scalar.activation(
                                    rho2[:], ps2[:], RELU,
                                    bias=B["b2_nb"][:, sqb:sqb + 1], scale=-1.0,
                                )
                                scrm = ep.tile([128, CHUNK], BF16, tag="scrm",
                                               name=f"scrm_{b}")
                                nc.scalar.activation(
                                    scrm[:], rho2[:], EXP, scale=-1.0,
                                    accum_out=RSM[:, col:col + 1],
                                )
                                scrr = ep.tile([128, CHUNK], BF16, tag="scrr",
                                               name=f"scrr_{b}")
                                nc.scalar.activation(
                                    scrr[:], ps2[:], RELU,
                                    bias=B["b2_d"][:, sqb:sqb + 1], scale=1.0,
                                    accum_out=RSR[:, col:col + 1],
                                )
                            else:
                                # v1-style exact blend: ScalarE Exp + DVE
                                # blend with fused row-sum accum (-> elu).
                                e2 = ep.tile([128, CHUNK], F32, tag="e2",
                                             name=f"e2_{b}")
                                nc.scalar.activation(
                                    e2[:], ps2[:], EXP,
                                    bias=B["b2_d"][:, sqb:sqb + 1], scale=1.0,
                                )
                                scr = ep.tile([128, CHUNK], F32R, tag="scr",
                                              name=f"scr_{b}")
                                nc.vector._custom_dve(
                                    BLEND_ACC, out=scr[:],
                                    accum_out=RS[:, col:col + 1],
                                    in0=ps2[:], in1=e2[:],
                                    s0=B["b2_d"][:, sqb:sqb + 1], s1=1.0,
                                )

                        n_done = 0
                        for e in range(q * 4, q * 4 + 4):
                            se = s * E + e
                            e4 = e % 4
                            # ---- layer 0a (first 128 features of member e)
                            ps0a = psp.tile([128, CHUNK], F32, tag="l0a", bufs=3)
                            for p in range(NP0):
                                nc.tensor.matmul(
                                    ps0a[:],
                                    w0at[:, p, :, e * 128:(e + 1) * 128],
                                    xt[:, p, :, cs],
                                    start=(p == 0),
                                    stop=(p == NP0 - 1),
                                    perf_mode=DR,
                                )
                            nc.vector._custom_dve(
                                POLY, out=G0[:, e4, :], in0=ps0a[:],
                                s0=B["b0a_c0"][:, se:se + 1],
                                s1=B["b0a_c1"][:, se:se + 1], imm2=KP,
                            )
                            # ---- layer 1: one hi DR + one e5m2 lo DR on the
                            # (g0a, g0b) plane pair
                            ps1 = psp.tile([H1, CHUNK], F32, tag="l1", bufs=2)
                            ifm = G0[:, e4:5:4 - e4, :]     # planes (e4, 4)
                            nc.tensor.matmul(
                                ps1[:], w1ht[:, :, e * 128:(e + 1) * 128], ifm,
                                start=True, stop=False, perf_mode=DR,
                            )
                            nc.tensor.matmul(
                                ps1[:], w1lt[:, :, e * 128:(e + 1) * 128], ifm,
                                start=False, stop=True, perf_mode=DR,
                            )
                            sl = _G1_SLOT[e4]
                            if e4 in SCAL1:
                                # exact elu+1 = r + m, pure ScalarE 3-pass
                                rho = ep.tile([H1, CHUNK], BF16, tag="rho",
                                              name=f"rho_{e}")
                                nc.scalar.activation(
                                    rho[:], ps1[:], RELU,
                                    bias=B["b1_nb"][:, se:se + 1], scale=-1.0,
                                )
                                nc.scalar.activation(
                                    G1[:, sl + 1, :], rho[:], EXP, scale=-1.0,
                                )
                                nc.scalar.activation(
                                    G1[:, sl, :], ps1[:], RELU,
                                    bias=B["b1_b"][:, se:se + 1], scale=1.0,
                                )
                            else:
                                nc.vector._custom_dve(
                                    POLY, out=G1[:, sl, :], in0=ps1[:],
                                    s0=B["b1_c0"][:, se:se + 1],
                                    s1=B["b1_c1"][:, se:se + 1], imm2=KP,
                                )
                            n_done += 1
                            if n_done >= 2:
                                do_l2_bank(n_done - 2)
            nc.sync.dma_start(rs_d[:], RS[:])
            nc.sync.dma_start(rsr_d[:], RSR[:])
            nc.sync.dma_start(rsm_d[:], RSM[:])
    nc.compile()
    _NC = nc
    return nc


# ------------------------------------------------------------- host side
def _q8(a):
    import ml_dtypes
    return np.clip(a, -240.0, 240.0).astype(ml_dtypes.float8_e4m3)


def _q5(a):
    import ml_dtypes
    return np.clip(a, -57344.0, 57344.0).astype(ml_dtypes.float8_e5m2)


def _prep_shared(w0, w1, w2, b0, b1, b2):
    """Pack rescaled weights/biases into the fp8 DoubleRow device layouts."""
    f = np.float32
    w0h_f = w0.astype(np.float64) / ALPHA                              # [S,E,384,160]
    b0e = b0[:, :, 0, :].astype(np.float64) / ALPHA                    # [S,E,160]
    b1e = b1[:, :, 0, :].astype(np.float64) / ALPHA - w1.astype(np.float64).sum(2)
    b2e = b2[:, :, 0, :].astype(np.float64) / ALPHA - w2.astype(np.float64).sum(2)

    # --- w0 hi/lo e4m3 planes
    w0hi = _q8(w0h_f).astype(np.float64)                               # [S,E,384,160]
    w0lo = _q8((w0h_f - w0hi) * LO_SCALE).astype(np.float64)
    sel = [[(w0hi, 0), (w0hi, 1)], [(w0hi, 2), (w0lo, 0)], [(w0lo, 1), (w0lo, 2)]]
    w0a = np.zeros((S, 128, NP0, 2, E * 128), dtype=f)
    w0b = np.zeros((S, 128, NP0, 2, NQ * 128), dtype=f)
    for p in range(NP0):
        for pl in range(2):
            arr, kt = sel[p][pl]
            blk = arr[:, :, kt * 128:(kt + 1) * 128, :]                # [S,E,128,160]
            w0a[:, :, p, pl, :] = (
                blk[..., :128].transpose(0, 2, 1, 3).reshape(S, 128, E * 128)
            )
            w0b[:, :, p, pl, :] = (
                blk[..., 128:H0].transpose(0, 2, 1, 3).reshape(S, 128, E * 32)
            )
    # --- w1 hi e4m3 + lo e5m2, planes (rows 0:128, padded tail)
    w1_64 = w1.astype(np.float64)
    w1hi_f = _q8(w1_64).astype(np.float64)
    w1lo_f = w1_64 - w1hi_f

    def pack_w1(arr):                                                  # [S,E,160,128]
        out = np.zeros((S, 128, 2, E * H1), dtype=np.float64)
        for e in range(E):
            out[:, :, 0, e * H1:(e + 1) * H1] = arr[:, e, :128, :]
            r0 = (e % 4) * 32
            out[:, r0:r0 + 32, 1, e * H1:(e + 1) * H1] = arr[:, e, 128:160, :]
        return out

    w1h = _q8(pack_w1(w1hi_f))
    w1l = _q5(pack_w1(w1lo_f))
    # --- w2 merged-bank planes, e4m3
    w2_64 = w2.astype(np.float64)
    w2p = np.zeros((S, 128, 2, NQ * 3 * 128), dtype=f)
    b2m = np.zeros((S, NQ, 3, 128), dtype=np.float64)
    for s in range(S):
        for q in range(NQ):
            for b in range(3):
                off = (q * 3 + b) * 128
                for piece, (mi, lo, hi, row) in enumerate(_L2_PIECES[b]):
                    e = 4 * q + mi
                    w2p[s, :, piece, off + row:off + row + hi - lo] = (
                        w2_64[s, e, :, lo:hi]
                    )
                    b2m[s, q, b, row:row + hi - lo] = b2e[s, e, lo:hi]
    w2p = _q8(w2p)

    def col_pack(b, lo, hi):
        return np.ascontiguousarray(b[:, :, lo:hi].reshape(S * E, hi - lo).T)

    b0a = col_pack(b0e, 0, 128)                                        # [128, S*E]
    b0b = np.ascontiguousarray(
        b0e[:, :, 128:].reshape(S, NQ, 4 * 32).transpose(2, 0, 1).reshape(128, S * NQ)
    )
    b1c = col_pack(b1e, 0, H1)                                         # [128, S*E]
    b2c = np.ascontiguousarray(b2m.reshape(S * NQ * 3, 128).T)

    shared = {
        "w0a": _q8(w0a), "w0b": _q8(w0b),
        "w1h": w1h, "w1l": w1l, "w2p": w2p,
        "b1_b": b1c.astype(f), "b1_nb": (-b1c).astype(f),
        "b2_d": b2c.astype(f), "b2_nb": (-b2c).astype(f),
    }
    for nm, b in (("b0a", b0a), ("b0b", b0b), ("b1", b1c)):
        shared[f"{nm}_c0"] = (1.0 + KP * b).astype(f)
        shared[f"{nm}_c1"] = (b + 1.0).astype(f)
    return shared


def _prep_core_x(aev_flat, idx_c):
    x = aev_flat[idx_c.reshape(-1)].reshape(S, A_SP, K0)     # [S,A_SP,384]
    xt = x.transpose(0, 2, 1)                                # [S,384,A_SP]
    xhi = _q8(xt).astype(np.float32)
    xlo = _q8(xt / LO_SCALE).astype(np.float32)
    xq = np.zeros((S, 128, NP0, 2, A_SP), dtype=np.float32)
    selx = [[(xhi, 0), (xhi, 1)], [(xhi, 2), (xlo, 0)], [(xlo, 1), (xlo, 2)]]
    for p in range(NP0):
        for pl in range(2):
            arr, kt = selx[p][pl]
            xq[:, :, p, pl, :] = arr[:, kt * 128:(kt + 1) * 128, :]
    return _q8(xq)


def _host_tail(results, w3, b3):
    """Blend banks: rs = row-sums of elu(u2).  ScalA banks: rs_r + rs_m =
    row-sums of elu(u2)+1 (subtract CHUNK).  Per-atom E = a*w3 . h2 + b3."""
    w3m = np.zeros((128, S, NQ, 3), dtype=np.float64)
    scala = np.zeros((S, NQ, 3), dtype=bool)
    scala[:, :, list(SCALA_BANKS)] = True
    for s in range(S):
        for q in range(NQ):
            for b in range(3):
                for (mi, lo, hi, row) in _L2_PIECES[b]:
                    w3m[row:row + hi - lo, s, q, b] = w3[s, 4 * q + mi, lo:hi, 0]
    w3rep = np.repeat(
        w3m.reshape(128, S * NQ * 3)[:, :, None], NCH, axis=2
    ).reshape(128, S * NQ * 3 * NCH)
    scala_rep = np.repeat(scala.reshape(S * NQ * 3)[:, None], NCH, axis=1
                          ).reshape(-1)[None, :]
    total = 0.0
    for cc in range(N_CORES):
        h2sum = np.where(
            scala_rep,
            results[cc]["rs_r"].astype(np.float64)
            + results[cc]["rs_m"].astype(np.float64) - CHUNK,
            results[cc]["rs"].astype(np.float64),
        )
        total += ALPHA * float((h2sum * w3rep).sum())
    total += float(b3.astype(np.float64).sum()) * (N_ATOMS // S)
    return np.array([total / E], dtype=np.float32)


def _run(inputs, trace=False, tmpdir=None):
    aev = np.asarray(inputs["aev"], dtype=np.float32)
    idx = np.asarray(inputs["idx"], dtype=np.int32)
    w3 = np.asarray(inputs["w3"], dtype=np.float32)
    b3 = np.asarray(inputs["b3"], dtype=np.float32)

    nc = _build_nc()
    shared = _prep_shared(
        np.asarray(inputs["w0"], dtype=np.float32),
        np.asarray(inputs["w1"], dtype=np.float32),
        np.asarray(inputs["w2"], dtype=np.float32),
        np.asarray(inputs["b0"], dtype=np.float32),
        np.asarray(inputs["b1"], dtype=np.float32),
        np.asarray(inputs["b2"], dtype=np.float32),
    )

    aev_flat = aev.reshape(-1, K0)
    in_maps = []
    for cc in range(N_CORES):
        idx_c = idx[:, cc * A_SP:(cc + 1) * A_SP]                # [S, A_SP]
        in_maps.append({"xq": _prep_core_x(aev_flat, idx_c), **shared})

    res = run_bass_kernel_spmd(
        nc, in_maps, core_ids=list(range(N_CORES)), trace=trace, tmpdir=tmpdir
    )
    out = _host_tail(res.results, w3, b3)
    return out, res


def kernel(**inputs):
    out, _ = _run(inputs, trace=bool(int(os.environ.get("BASS_KERNEL_TRACE", "0"))))
    return out
